# revision 45
# baseline (speedup 1.0000x reference)
"""Trainium2 Bass kernel for nn_BertEncoder_403726926494.

Reference computation (per batch element):
  - ragged sentence extraction from hidden_states, masked-softmax attention
    pooling per sentence with W_doc            -> doc_pooled [B, D, H]
  - query extraction (rows 1..32), masked-softmax pooling with W_query
    broadcast over D                           -> q_bcast   [B, D, H]

Device strategy (SPMD, one program on 8 cores, 8 batch elements per core):
  - All float traffic staged host-side as bf16 (tolerance is 2e-2; bf16
    keeps us ~5x under it).  PE matmuls run 4x faster than f32, DMA moves
    half the bytes.
  - The 8 per-core examples (slots) are concatenated into ONE dense token
    stream and chunked into [128, 769] tiles (768 hidden + a baked ones
    column for the softmax denominator).  8 slots x 16 sentences = 128
    selector columns = one full-width PE stationary.  Per chunk:
      score  s[t] = x_t . W_doc      one fused DVE scalar_tensor_tensor
                                     with accum_out (single pass)
      alphaU[t,c] = exp(s[t]+sel)    one ACT op against a host-built
                                     {0,-1e30} selector [128 cols]
      num|den += alphaU^T @ [X|1]    one PE matmul pair (N=512 + N=257)
                                     accumulating over all 13 chunks
  - out[16s+j] = num/(den+eps); one [128,768] scale, one contiguous
    doc_out DMA.  Empty sentences stay 0.
  - Query path: all 8 examples' rows 1..32 pack into 2 chunks of 128;
    an M=8 stationary pools every example in one 2-matmul chain.
  - W rows are staged pre-broadcast to 128 partitions.  b_doc / b_query
    shift every score in a softmax segment equally and cancel; ignored.
  - Outputs return as bf16 and are upcast on the host.

Hardware notes (learned the hard way):
  - tensor_tensor_reduce faults the device (NRT unrecoverable); DVE
    scalar_tensor_tensor with accum_out is the working fused op.
  - scalar_tensor_tensor is not supported by codegen on GpSimd.
  - dma_start may only issue from sync/scalar/gpsimd queues.
"""

import numpy as np
import ml_dtypes

B, L, H = 64, 512, 768
D, S, Q = 16, 64, 32
NCORES = 8
SLOTS = 8
HP = H + 1  # ones column appended
NEG_BIAS = -1.0e30
DEN_EPS = 1.0e-30
BF16 = ml_dtypes.bfloat16

# Per-score-op engine (chunks then queries): "dstt" (fused DVE op),
# "dtr" (DVE TT + DVE reduce), "gact" (GpSimd TT + ACT accum reduce)
# All doc chunks on the fused DVE op: offloading any to a cross-engine
# reduce path (GpSimd or ACT) measurably inflates concurrent DVE ops via
# SBUF contention and nets out slower.  Queries alone go DVE-TT+ACT-reduce.
SCORE_ENG_DOC = ["dstt"] * 32
DACT_SLOT: dict = {}
SCORE_ENG_Q = ["dact", "dact"]
# x stream DMA slicing (chunks per slice)
# Uniform 2-plane slices: DMA delivers at ~DVE consumption rate, so fine
# granularity minimizes all-or-nothing stalls at slice boundaries.
SLICE_PAT = [2, 2, 2, 2, 2, 2, 2, 2]  # first slice = W plane + chunk 0
# Emit query score ops after this chunk index
Q_AFTER = 3
BUILD_STAGE = 3  # debug: 0=DMA only, 1=+scores, 2=+exp, 3=full

_compiled: dict = {}


def _geometry(slot_spans):
    offs = [0]
    for sp in slot_spans:
        offs.append(offs[-1] + sp)
    tot = offs[-1]
    nck = (tot + 127) // 128
    return offs, tot, nck


def _slices(nck):
    out = []
    c = 0
    pat = list(SLICE_PAT)
    while c < nck:
        n = min(pat.pop(0) if pat else 3, nck - c)
        out.append((c, n))
        c += n
    return out


def _build(slot_spans):
    """Build + compile the SPMD Bass program for the given per-slot spans."""
    from contextlib import ExitStack

    import concourse.bacc as bacc
    import concourse.tile as tile
    from concourse import mybir

    f32 = mybir.dt.float32
    bf16 = mybir.dt.bfloat16
    MULT = mybir.AluOpType.mult
    ADD = mybir.AluOpType.add
    EXP = mybir.ActivationFunctionType.Exp
    COPY = mybir.ActivationFunctionType.Copy

    offs, tot, nck = _geometry(slot_spans)
    slices = _slices(nck + 1)  # planes: 0 = W_doc broadcast, 1.. = chunks

    nc = bacc.Bacc(
        "TRN2", target_bir_lowering=False, debug=False, num_devices=NCORES
    )
    xs = nc.dram_tensor("xs", [128, nck + 1, HP], bf16, kind="ExternalInput").ap()
    selt = nc.dram_tensor("selt", [128, nck, 128], bf16, kind="ExternalInput").ap()
    qstage = nc.dram_tensor("qstage", [128, 2, HP], bf16, kind="ExternalInput").ap()
    qmask = nc.dram_tensor("qmask", [128, 2, 8], bf16, kind="ExternalInput").ap()
    wbq = nc.dram_tensor("wbq", [128, H], bf16, kind="ExternalInput").ap()
    doc_out = nc.dram_tensor(
        "doc_out", [SLOTS * D, H], bf16, kind="ExternalOutput"
    ).ap()
    q_out = nc.dram_tensor("q_out", [SLOTS, H], bf16, kind="ExternalOutput").ap()

    with tile.TileContext(nc) as tc, ExitStack() as ctx:
        const = ctx.enter_context(tc.tile_pool(name="const", bufs=1))
        xpool = ctx.enter_context(tc.tile_pool(name="xp", bufs=1))
        apool = ctx.enter_context(tc.tile_pool(name="apl", bufs=4))
        work = ctx.enter_context(tc.tile_pool(name="work", bufs=1))
        nump = ctx.enter_context(tc.tile_pool(name="nump", bufs=1, space="PSUM"))

        # ---- input DMAs: first x slice and W first (they gate the first
        # score op); remaining x slices next; metadata by first use.  DMA
        # engines process descriptors in rough global issue order, so
        # emission order here is completion priority.
        xts = []

        def xslice(i, eng):
            c0, n = slices[i]
            t = xpool.tile([128, n, HP], bf16, tag=f"x{c0}", name=f"x{c0}")
            eng.dma_start(out=t[:], in_=xs[:, c0 : c0 + n, :])
            xts.append((c0, n, t))

        selt_t = const.tile([128, nck, 128], bf16)
        wbq_t = const.tile([128, H], bf16)
        qp_t = const.tile([128, 2, HP], bf16)
        qmask_t = const.tile([128, 2, 8], bf16)

        # the whole x stream outranks metadata: selt is first needed at
        # exp(c0) (~1 score-op after the stream head), qstage at Q_AFTER
        for i in range(len(slices)):
            xslice(i, [nc.sync, nc.gpsimd][i % 2])
        wbd_t = xts[0][2][:, 0, 0:H]  # W_doc broadcast rides as plane 0
        nc.sync.dma_start(out=selt_t[:], in_=selt[:])
        nc.gpsimd.dma_start(out=qp_t[:], in_=qstage[:])
        nc.sync.dma_start(out=qmask_t[:], in_=qmask[:])
        nc.gpsimd.dma_start(out=wbq_t[:], in_=wbq[:])


        def xchunk(c):
            p = c + 1  # plane index (plane 0 is W_doc)
            for c0, n, t in xts:
                if c0 <= p < c0 + n:
                    return t[:, p - c0, :]
            raise AssertionError

        # ---- per-chunk score -> exp -> matmul ----
        scol = work.tile([128, nck], f32, tag="scol", name="scol")
        qscol = work.tile([128, 2], f32, tag="qscol", name="qscol")
        junk_dve = work.tile([128, H], bf16, tag="jd", name="jd")
        junk_gps = work.tile([128, H], bf16, tag="jg", name="jg")
        junk_q = work.tile([128, 5, H], bf16, tag="jq", name="jq")
        junk2 = work.tile([128, H], bf16, tag="j2", name="j2")

        def emit_score(x_ap, scol_ap, eng, wb, qslot=0):
            if eng == "dstt":
                nc.vector.scalar_tensor_tensor(
                    out=junk_dve[:], in0=x_ap, scalar=1.0, in1=wb,
                    op0=MULT, op1=MULT, accum_out=scol_ap,
                )
            elif eng == "dact":
                # DVE multiply (bf16 2x mode) + ACT accumulate-reduce; own
                # scratch so the next DVE op doesn't WAR-stall on ACT's read
                j = junk_q[:, qslot, :]
                nc.vector.tensor_tensor(out=j, in0=x_ap, in1=wb, op=MULT)
                nc.scalar.activation(
                    junk2[:], j, COPY, bias=0.0, scale=1.0,
                    accum_out=scol_ap,
                )
            elif eng == "dtr":
                nc.vector.tensor_tensor(
                    out=junk_dve[:], in0=x_ap, in1=wb, op=MULT
                )
                nc.vector.tensor_reduce(
                    out=scol_ap, in_=junk_dve[:],
                    axis=mybir.AxisListType.X, op=ADD,
                )
            else:  # gact
                nc.gpsimd.tensor_tensor(
                    out=junk_gps[:], in0=x_ap, in1=wb, op=MULT
                )
                nc.scalar.activation(
                    junk2[:], junk_gps[:], COPY, bias=0.0, scale=1.0,
                    accum_out=scol_ap,
                )

        numg = nump.tile([128, 1024], f32, tag="num", name="num")
        qnum = nump.tile([8, 1024], f32, tag="qnum", name="qnum")

        def emit_chunk(c):
            x = xchunk(c)
            if BUILD_STAGE < 1:
                return
            emit_score(
                x[:, 0:H], scol[:, c : c + 1], SCORE_ENG_DOC[c], wbd_t,
                qslot=DACT_SLOT.get(c, 0),
            )
            if BUILD_STAGE < 2:
                return
            at = apool.tile([128, 128], bf16, tag="at", name=f"at{c}")
            nc.scalar.activation(
                at[:], selt_t[:, c, :], EXP, bias=scol[:, c : c + 1], scale=1.0
            )
            if BUILD_STAGE < 3:
                return
            first, last = c == 0, c == nck - 1
            nc.tensor.matmul(
                numg[:, 0:512], at[:], x[:, 0:512],
                start=first, stop=last, skip_group_check=True,
            )
            nc.tensor.matmul(
                numg[:, 512:HP], at[:], x[:, 512:HP],
                start=first, stop=last, skip_group_check=True,
            )

        def emit_query(b):
            emit_score(
                qp_t[:, b, 0:H], qscol[:, b : b + 1], SCORE_ENG_Q[b], wbq_t[:],
                qslot=b,
            )
            qat = apool.tile([128, 8], bf16, tag="qat", name=f"qat{b}")
            nc.scalar.activation(
                qat[:], qmask_t[:, b, :], EXP, bias=qscol[:, b : b + 1], scale=1.0
            )
            nc.tensor.matmul(
                qnum[:, 0:512], qat[:], qp_t[:, b, 0:512],
                start=b == 0, stop=b == 1, skip_group_check=True,
            )
            nc.tensor.matmul(
                qnum[:, 512:HP], qat[:], qp_t[:, b, 512:HP],
                start=b == 0, stop=b == 1, skip_group_check=True,
            )

        for c in range(nck):
            emit_chunk(c)
            if c == Q_AFTER and BUILD_STAGE >= 3:
                emit_query(0)
                emit_query(1)

        if BUILD_STAGE >= 3:
            # ---- finish: doc scale on ACT, query scale on DVE (parallel) ----
            # query finish first (its matmul chain stops early, and ACT has
            # slack mid-phase while DVE is the bottleneck)
            qde = work.tile([8, 1], f32, tag="qde", name="qde")
            nc.vector.tensor_scalar(
                out=qde[:], in0=qnum[:, H:HP], scalar1=DEN_EPS, scalar2=None,
                op0=ADD,
            )
            qrec = work.tile([8, 1], f32, tag="qrec", name="qrec")
            nc.vector.reciprocal(qrec[:], qde[:])
            qo = work.tile([8, H], bf16, tag="qo", name="qo")
            nc.scalar.activation(
                qo[:], qnum[:, 0:H], COPY, bias=0.0, scale=qrec[:, 0:1]
            )
            nc.scalar.dma_start(out=q_out[:], in_=qo[:])

            # doc finish: halves scaled in parallel on ACT and DVE, two
            # output DMAs from different queues
            de = work.tile([128, 1], f32, tag="de", name="de")
            nc.vector.tensor_scalar(
                out=de[:], in0=numg[:, H:HP], scalar1=DEN_EPS, scalar2=None,
                op0=ADD,
            )
            rec = work.tile([128, 1], f32, tag="rec", name="rec")
            nc.vector.reciprocal(rec[:], de[:])
            do = work.tile([128, H], bf16, tag="do", name="do")
            HH = H // 2
            nc.scalar.activation(
                do[:, 0:HH], numg[:, 0:HH], COPY, bias=0.0, scale=rec[:, 0:1]
            )
            nc.vector.tensor_scalar(
                out=do[:, HH:H], in0=numg[:, HH:H], scalar1=rec[:, 0:1],
                scalar2=None, op0=MULT,
            )
            nc.sync.dma_start(out=doc_out[:, 0:HH], in_=do[:, 0:HH])
            nc.gpsimd.dma_start(out=doc_out[:, HH:H], in_=do[:, HH:H])
        else:
            zo = work.tile([128, H], bf16, tag="zo", name="zo")
            nc.vector.memset(zo[:], 0.0)
            nc.sync.dma_start(out=doc_out[:, :], in_=zo[:])
            nc.scalar.dma_start(out=q_out[:], in_=zo[0:SLOTS, :])

    nc.compile()
    return nc


def _prepare(query_len, seq_lens):
    """Host-side geometry: spans, slot assignment (rank-sorted)."""
    ql = np.asarray(query_len).astype(np.int64)
    sl = np.asarray(seq_lens).astype(np.int64)
    offs = ql[:, None] + 2 + np.cumsum(sl, axis=1) - sl  # [B, D] sentence starts
    end = ql + 2 + sl.sum(axis=1)
    span = np.maximum(end, 1 + Q)  # query rows 1..32 must be covered
    order = np.argsort(-span, kind="stable")  # rank -> example id
    slot_spans = tuple(int(span[order[8 * s]]) for s in range(SLOTS))
    ex_map = np.empty((NCORES, SLOTS), np.int64)
    for c in range(NCORES):
        for s in range(SLOTS):
            ex_map[c, s] = int(order[8 * s + c])
    return slot_spans, ex_map, (offs, ql, sl)


def _stage_core(hs, c, slot_spans, ex_map, geo, wrow):
    soffs, tot, nck = _geometry(slot_spans)
    offs, ql, sl = geo
    xs32 = np.zeros((128, nck + 1, HP), np.float32)
    xs32[:, 0, 0:H] = wrow[None, :]
    selt32 = np.full((128, nck, 128), NEG_BIAS, np.float32)
    qstage32 = np.zeros((128, 2, HP), np.float32)
    qmask32 = np.full((128, 2, 8), NEG_BIAS, np.float32)
    xs32[:, :, H] = 1.0
    qstage32[:, :, H] = 1.0

    for s in range(SLOTS):
        e = int(ex_map[c, s])
        spn = slot_spans[s]
        qoff = soffs[s]
        idx = np.arange(qoff, qoff + spn)
        xs32[idx % 128, idx // 128 + 1, 0:H] = hs[e, 0:spn, :]
        for j in range(D):
            ln = int(sl[e, j])
            if ln == 0:
                continue
            o = int(offs[e, j])
            t = np.arange(qoff + o, qoff + o + ln)
            selt32[t % 128, t // 128, 16 * s + j] = 0.0
        b, k = divmod(s, 4)
        qstage32[32 * k : 32 * k + 32, b, 0:H] = hs[e, 1 : 1 + Q, :]
        qmask32[32 * k + np.arange(int(ql[e])), b, s] = 0.0
    return {
        "xs": xs32.astype(BF16),
        "selt": selt32.astype(BF16),
        "qstage": qstage32.astype(BF16),
        "qmask": qmask32.astype(BF16),
    }


def kernel(hidden_states, W_doc, b_doc, W_query, b_query, query_len, seq_lens):
    hs = np.ascontiguousarray(np.asarray(hidden_states, dtype=np.float32))
    wd = np.asarray(W_doc, np.float32).reshape(H)
    wq = np.asarray(W_query, np.float32).reshape(H)

    slot_spans, ex_map, geo = _prepare(query_len, seq_lens)

    nc = _compiled.get(slot_spans)
    if nc is None:
        nc = _build(slot_spans)
        _compiled[slot_spans] = nc

    wbq = np.broadcast_to(wq[None, :], (128, H)).astype(BF16)
    in_maps = []
    for c in range(NCORES):
        m = _stage_core(hs, c, slot_spans, ex_map, geo, wd)
        m["wbq"] = wbq
        in_maps.append(m)

    from concourse.bass_utils import run_bass_kernel_spmd

    res = run_bass_kernel_spmd(nc, in_maps, list(range(NCORES)))

    doc = np.empty((B, D, H), np.float32)
    qp = np.empty((B, H), np.float32)
    for c in range(NCORES):
        r = res.results[c]
        dout = np.asarray(r["doc_out"], dtype=np.float32).reshape(SLOTS, D, H)
        qout = np.asarray(r["q_out"], dtype=np.float32)
        for s in range(SLOTS):
            e = int(ex_map[c, s])
            doc[e] = dout[s]
            qp[e] = qout[s]
    q_bcast = np.broadcast_to(qp[:, None, :], (B, D, H))
    return doc, q_bcast
